# revision 2
# baseline (speedup 1.0000x reference)
"""DIN (Deep Interest Network) Trainium2 kernel — 8-core data-parallel.

Problem: nn_DIN_49383533969756.
  B=2048, L=200 history, 3 item-feature tables [1000,32], user table [50000,64],
  linear attention (fc1/fc2 with no nonlinearity), masked weighted pooling,
  then a 256->200->80->1 MLP with Dice (full-batch batchnorm stats) gates.

Key algebraic collapse: the attention MLP is linear, so
  score[b,l] = c[b] + ub[b,l] . v[b]
with w_eff = w_att2 @ w_att1 split into (wq|wu|wd|wm) per 96-dim chunk:
  v[b] = (wu - wd) + item_emb[b] * wm   (elementwise per feature chunk)
  c[b] = item_emb[b] . (wq + wd) + (w_att2 @ b_att1 + b_att2)

Per-core layout: batch shard of 256 rows as (partition p, hi) -> b = p + 128*hi.
History embeddings are fetched with SWDGE dma_gather using 64B descriptors
(bf16 rows, elem_size=32 at 256B row stride) from a combined bf16 table
[3008, 128] in HBM — the 256B-elem_size bass assert is bypassed via
dma_gather_raw; the Q7 ucode and SDMA handle 64B packets fine. The index
stream interleaves the 3 item features, so one 12800-idx gather (Q7 scratch
caps num_idxs at ~16k) fills a third of a [128, 300, 32] bf16 tile whose
free dim is (l, hi, k), giving 96-wide contiguous rows per (b, l) for
single-instruction score/browse DVE ops. The i1==0 mask is folded into the
gather by remapping masked indices (all 3 features) to an all-zero table row.
Score mult runs in bf16 (2x DVE); reduces accumulate to f32.

Dice batch statistics are exact: per-shard (sum, sumsq) are AllReduced across
the 8 cores mid-kernel (per-shard stats alone land at 4.5e-2 rel err > 2e-2).
"""

import numpy as np
from contextlib import ExitStack

import concourse.bacc as bacc
import concourse.bass as bass
import concourse.mybir as mybir
import concourse.tile as tile
from concourse import library_config
from concourse.bass_utils import run_bass_kernel_spmd
import concourse.tile_sem_assignment as _tsa
import concourse.mybir as _mybir

# Tile assigns SWDGE DMA completion-sem lanes round-robin, which breaks the
# one-queue-per-sem FIFO invariant when dma_gather uses queue_num > 0. Pin
# lane = queue_num so each SWDGE queue owns a dedicated sem lane.
if not getattr(_tsa.TileClockTick, "_swdge_queue_lane_patch", False):
    _orig_assign_tick = _tsa.TileClockTick._assign_tick

    def _assign_tick_queue_lanes(self, inst):
        if (inst.engine == _mybir.EngineType.Pool
                and isinstance(inst, _tsa.DMAInst)):
            self.next_sw_dma_idx = int(getattr(inst, "queue_num", 0) or 0)
        return _orig_assign_tick(self, inst)

    _tsa.TileClockTick._assign_tick = _assign_tick_queue_lanes
    _tsa.TileClockTick._swdge_queue_lane_patch = True

F32 = mybir.dt.float32
BF16 = mybir.dt.bfloat16
I16 = mybir.dt.int16
ALU = mybir.AluOpType
AX = mybir.AxisListType
ACT = mybir.ActivationFunctionType

# problem dims (hardcoded per contract)
B, L = 2048, 200
DF = 32              # per item-feature embedding dim
E = 96               # item embedding (3 chunks of 32)
U = 64               # user embedding
H1, H2 = 200, 80
V_ITEM = 1000
V_USER = 50000
EPS = 1e-8

N_CORES = 8
BS = B // N_CORES    # 256 batch rows per core
P = 128
HI = BS // P         # 2
LCH = 40             # l values per gather chunk
NCH = L // LCH       # 8 chunks
BLK = 2 * LCH        # (l, hi) blocks per chunk
assert LCH % 2 == 0, "browse half-add drops an l slot if LCH is odd"
NIDX = BLK * P       # 6400 gather indices per (chunk, feature)
ES = 64              # padded table row, f32 elems (256B)
HS = 100             # H1 split rows per matmul (2 splits)


def _ap0(a, extra):
    """Append a step-0 broadcast dim of size `extra` to an AP."""
    return bass.AP(tensor=a.tensor, offset=a.offset, ap=list(a.ap) + [[0, extra]])


def dma_gather_raw(gp, out_ap, in_ap, idxs_ap, num_idxs, elem_size, elem_step,
                   single_packet=True, queue_num=0):
    """bass.BassGpSimd.dma_gather (non-transpose, HBM source) without the
    256B elem_size restriction. The Q7 ucode emits packets of elem_size
    bytes at stride elem_step; only the row *stride* must be a multiple of
    256B (encoded /256 in the instruction). Lets us gather 128B rows from a
    256B-stride table, halving gathered bytes."""
    import concourse.mybir as _mb
    from concourse import ap_utils as _apu
    assert idxs_ap.dtype == _mb.dt.int16
    assert in_ap.dtype == out_ap.dtype
    elem_size_bytes = elem_size * _mb.dt.size(in_ap.dtype)
    assert elem_size_bytes % 64 == 0
    assert _apu.ap_is_contiguous(out_ap.ap[1:])
    assert _apu.ap_is_contiguous(idxs_ap.ap[1:])
    assert in_ap.ap[-1][1] == out_ap.ap[-1][1] == elem_size
    assert out_ap.ap[0][1] * out_ap.ap[1][1] == ((num_idxs + 127) // 128) * 128
    assert in_ap.ap[0][0] == elem_step
    stride_bytes = elem_step * _mb.dt.size(in_ap.dtype)
    assert stride_bytes % 256 == 0 and stride_bytes // 256 < 256
    _in_ap = gp.lower_ap_dma(in_ap, for_custom_bir_dma=True)
    inst = gp.add_instruction(
        _mb.InstDMAGatherAnt(
            name=gp.bass.get_next_instruction_name(),
            ins=[*_in_ap, gp.lower_ap(idxs_ap),
                 gp.lower_val_access(gp.to_reg(num_idxs))],
            outs=[gp.lower_ap(out_ap)],
            transpose=False,
            num_idxs=num_idxs,
            elem_size=elem_size,
            stride_bytes_256=stride_bytes // 256,
            gen_mode=0,
            single_packet=single_packet,
            queue_num=queue_num,
            sbuf_tokens_per_rank=0,
            sbuf_free_dim_per_rank=0,
            sbuf_free_dim_pad_per_rank=0,
            sbuf_byte_offset=0,
        )
    )
    return inst


def _bcast_col(a, n):
    """[P,1]-shaped AP -> [P,n] via step-0 broadcast of the single column."""
    assert a.ap[-1][1] == 1, a.ap
    return bass.AP(tensor=a.tensor, offset=a.offset, ap=list(a.ap[:-1]) + [[0, n]])


def build_nc(n_cores=N_CORES, debug=False, use_cc=True, variant="full", reps=1):
    nc = bacc.Bacc("TRN2", num_devices=n_cores, num_swdge_queues=4,
                   dynamic_dma_scratch_size=1 << 15)

    tab = nc.dram_tensor("tab", [3 * V_ITEM, ES], F32, kind="ExternalInput")
    tab16 = nc.dram_tensor("tab16", [3 * V_ITEM + 8, 2 * ES], BF16, kind="ExternalInput")
    tu = nc.dram_tensor("tu", [V_USER, U], F32, kind="ExternalInput")
    g_idx = nc.dram_tensor("g_idx", [P, 3 * L * P * HI // 16], I16, kind="ExternalInput")
    q_idx = nc.dram_tensor("q_idx", [P, 3, BS // 16], I16, kind="ExternalInput")
    u_lo = nc.dram_tensor("u_lo", [P, BS // 16], I16, kind="ExternalInput")
    u_hi = nc.dram_tensor("u_hi", [P, BS // 16], I16, kind="ExternalInput")
    u_sel = nc.dram_tensor("u_sel", [P, HI], F32, kind="ExternalInput")
    watt = nc.dram_tensor("watt", [1, 3 * E], F32, kind="ExternalInput")
    beff = nc.dram_tensor("beff", [1, 2], F32, kind="ExternalInput")
    w1t = nc.dram_tensor("w1t", [P, 2, H1], F32, kind="ExternalInput")
    dice1 = nc.dram_tensor("dice1", [P, 2, 4], F32, kind="ExternalInput")
    w2t = nc.dram_tensor("w2t", [P, 2, H2], F32, kind="ExternalInput")
    dice2 = nc.dram_tensor("dice2", [P, 4], F32, kind="ExternalInput")
    w3t = nc.dram_tensor("w3t", [P, 1], F32, kind="ExternalInput")
    ident_in = nc.dram_tensor("ident", [P, P], F32, kind="ExternalInput")
    y_out = nc.dram_tensor("y", [1, BS], F32, kind="ExternalOutput")
    if debug:
        dbg_s = nc.dram_tensor("dbg_s", [P, HI, L], F32, kind="ExternalOutput")
        dbg_feat = nc.dram_tensor("dbg_feat", [P, HI, 256], F32, kind="ExternalOutput")
        dbg_x1 = nc.dram_tensor("dbg_x1", [HS, 2, BS], F32, kind="ExternalOutput")
        dbg_x1d = nc.dram_tensor("dbg_x1d", [HS, 2, BS], F32, kind="ExternalOutput")
        dbg_st1 = nc.dram_tensor("dbg_st1", [P, 4], F32, kind="ExternalOutput")
        dbg_red1 = nc.dram_tensor("dbg_red1", [P, 4], F32, kind="ExternalOutput")

    cc1_in = nc.dram_tensor("cc1_in", [P, 4], F32, kind="Internal")
    cc1_out = nc.dram_tensor("cc1_out", [P, 4], F32, kind="Internal", addr_space="Shared")
    cc2_in = nc.dram_tensor("cc2_in", [P, 2], F32, kind="Internal")
    cc2_out = nc.dram_tensor("cc2_out", [P, 2], F32, kind="Internal", addr_space="Shared")
    groups = [list(range(n_cores))]

    with tile.TileContext(nc, num_cores=n_cores) as tc, ExitStack() as ctx:
        ones = ctx.enter_context(tc.tile_pool(name="ones", bufs=1))
        small = ctx.enter_context(tc.tile_pool(name="small", bufs=2))
        gpool = ctx.enter_context(tc.tile_pool(name="gpool", bufs=3))
        ipool = ctx.enter_context(tc.tile_pool(name="ipool", bufs=3))
        ppool = ctx.enter_context(tc.tile_pool(name="ppool", bufs=2))
        psum = ctx.enter_context(tc.tile_pool(name="psum", bufs=2, space="PSUM"))

        nc.gpsimd.load_library(library_config.mlp)

        for rep_i in range(reps):
            # --- prefetch first history gathers (longest pole) ---
            NC3 = 3 * NIDX // 16
            HB = 3 * BLK // 2
            HN = 3 * NIDX // 2
            gathered = {}

            def emit_gather(ci):
                # 3 sub-gathers of NIDX=12800 idx: the Q7 dma_gather ucode
                # stages idx as int32 in its 64KB scratch, so num_idxs must
                # stay <= ~16k per instruction.
                it = ipool.tile([P, NC3], I16, tag="gidx", name="it")
                nc.sync.dma_start(out=it, in_=g_idx[:, ci * NC3:(ci + 1) * NC3])
                u_all = gpool.tile([P, 3 * BLK, DF], BF16, tag="u_all",
                                   name="u_all")
                for j in range(3):
                    dma_gather_raw(
                        nc.gpsimd, u_all[:, j * BLK:(j + 1) * BLK, :],
                        tab16[:, 0:DF], it[:, j * (NC3 // 3):(j + 1) * (NC3 // 3)],
                        NIDX, DF, 2 * ES, single_packet=False,
                        queue_num=(3 * ci + j + 1) % 4)
                gathered[ci] = u_all

            assert variant != "nogather"
            # --- item embeddings (q) + user embedding gathers (tiny; queue 0
            # pair) emitted first so the big desc-gens don't block them ---
            qk = []
            qis = []
            for k in range(3):
                qi = small.tile([P, BS // 16], I16, tag="qidx")
                nc.sync.dma_start(out=qi, in_=q_idx[:, k, :])
                qis.append(qi)
            uli = small.tile([P, BS // 16], I16, tag="uidx", name="uli")
            nc.sync.dma_start(out=uli, in_=u_lo[:, :])
            uhi = small.tile([P, BS // 16], I16, tag="uidx2", name="uhi")
            nc.sync.dma_start(out=uhi, in_=u_hi[:, :])
            for k in range(3):
                q_t = ones.tile([P, HI, ES], F32, tag=f"q{k}")
                nc.gpsimd.dma_gather(q_t[:], tab[:, :], qis[k][:], BS, BS, ES)
                qk.append(q_t)
            ulo_t = small.tile([P, HI, U], F32, tag="ulo")
            nc.gpsimd.dma_gather(ulo_t[:], tu[:, :], uli[:], BS, BS, U)
            uhi_t = small.tile([P, HI, U], F32, tag="uhi")
            nc.gpsimd.dma_gather(uhi_t[:], tu[32768:V_USER, :], uhi[:], BS, BS, U)
            emit_gather(0)
            emit_gather(1)

            if rep_i == 0:
                # constants: emitted after the idx DMAs so HWDGE serves the
                # latency-critical gather path first
                wa = ones.tile([P, 3 * E], F32, name="wa")
                nc.sync.dma_start(out=wa, in_=watt[0:1, :].partition_broadcast(P)[:, 0, :])
                be = ones.tile([P, 2], F32, name="be")
                nc.sync.dma_start(out=be, in_=beff[0:1, :].partition_broadcast(P)[:, 0, :])
                usel_t = ones.tile([P, HI], F32, name="usel_t")
                nc.sync.dma_start(out=usel_t, in_=u_sel[:, :])
                ident = ones.tile([P, P], F32, name="ident")
                nc.sync.dma_start(out=ident, in_=ident_in[:, :])
                w1t_t = ones.tile([P, 2, H1], F32, name="w1t_t")
                nc.sync.dma_start(out=w1t_t, in_=w1t[:, :, :])
                d1_t = ones.tile([P, 2, 4], F32, name="d1_t")
                nc.sync.dma_start(out=d1_t, in_=dice1[:, :, :])
                w2t_t = ones.tile([P, 2, H2], F32, name="w2t_t")
                nc.sync.dma_start(out=w2t_t, in_=w2t[:, :, :])
                d2_t = ones.tile([P, 4], F32, name="d2_t")
                nc.sync.dma_start(out=d2_t, in_=dice2[:, :])
                w3t_t = ones.tile([P, 1], F32, name="w3t_t")
                nc.sync.dma_start(out=w3t_t, in_=w3t[:, :])

            # user = lo + (hi - lo) * sel
            ud = small.tile([P, HI, U], F32, tag="ud")
            nc.vector.tensor_tensor(out=ud[:], in0=uhi_t[:], in1=ulo_t[:], op=ALU.subtract)
            selb = bass.AP(tensor=usel_t.tensor, offset=usel_t[:].offset,
                           ap=[usel_t[:].ap[0], [1, HI], [0, U]])
            nc.vector.tensor_tensor(out=ud[:], in0=ud[:], in1=selb, op=ALU.mult)
            user_t = ones.tile([P, HI, U], F32)
            nc.vector.tensor_tensor(out=user_t[:], in0=ud[:], in1=ulo_t[:], op=ALU.add)

            # --- v[b] and c[b] ---
            vk = []
            c_t = ones.tile([P, HI], F32)
            cscr = small.tile([P, HI, DF], F32, tag="cscr")
            ck = [small.tile([P, HI], F32, tag=f"ck{k}", name=f"ck{k}") for k in range(3)]
            for k in range(3):
                wm_b = bass.AP(tensor=wa.tensor, offset=wa[:, DF * k:DF * k + DF].offset,
                               ap=[wa[:].ap[0], [0, HI], [1, DF]])
                wud_b = bass.AP(tensor=wa.tensor, offset=wa[:, E + DF * k:E + DF * k + DF].offset,
                                ap=[wa[:].ap[0], [0, HI], [1, DF]])
                wqd_b = bass.AP(tensor=wa.tensor, offset=wa[:, 2 * E + DF * k:2 * E + DF * k + DF].offset,
                                ap=[wa[:].ap[0], [0, HI], [1, DF]])
                v_t = ones.tile([P, HI, DF], F32, tag=f"v{k}")
                nc.vector.tensor_tensor(out=v_t[:], in0=qk[k][:, :, 0:DF], in1=wm_b, op=ALU.mult)
                nc.vector.tensor_tensor(out=v_t[:], in0=v_t[:], in1=wud_b, op=ALU.add)
                vk.append(v_t)
                nc.vector.tensor_tensor(out=cscr[:], in0=qk[k][:, :, 0:DF], in1=wqd_b, op=ALU.mult)
                nc.vector.tensor_reduce(out=ck[k][:], in_=cscr[:], axis=AX.X, op=ALU.add)
            v_all = ones.tile([P, HI, 3 * DF], BF16, tag="v_all", name="v_all")
            for k in range(3):
                nc.vector.tensor_copy(out=v_all[:, :, DF * k:DF * k + DF], in_=vk[k][:])
            nc.vector.tensor_tensor(out=c_t[:], in0=ck[0][:], in1=ck[1][:], op=ALU.add)
            nc.vector.tensor_tensor(out=c_t[:], in0=c_t[:], in1=ck[2][:], op=ALU.add)
            beb = _bcast_col(be[:, 0:1], HI)
            nc.vector.tensor_tensor(out=c_t[:], in0=c_t[:], in1=beb, op=ALU.add)

            # --- main loop: gather ub, score, weighted browse accumulation ---
            # All reductions run as bf16 half-add trees (DVE 2x) instead of
            # 1x tensor_reduce passes; browse partials accumulate in a bf16
            # [P, HI, 5, 96] tile, fully reduced once after the loop.
            bacc16 = ones.tile([P, HI, 5, 3 * DF], BF16, tag="bacc16",
                               name="bacc16")

            def tree_view(t, width, off_elems=0):
                """[P, HI, LCH, width] view of tile t at innermost offset."""
                ta = t[:]
                return bass.AP(tensor=ta.tensor, offset=ta.offset + off_elems,
                               ap=[ta.ap[0], [LCH * 3 * DF, HI],
                                   [3 * DF, LCH], [1, width]])

            for ci in range(2, NCH):
                emit_gather(ci)
            for ci in range(NCH):
                u_all = gathered[ci]
                if variant == "gather":
                    continue
                in0 = u_all[:, :, :].rearrange(
                    "p (l two three) f -> p two l (three f)", two=2, three=3)
                scr = ppool.tile([P, HI, LCH, 3 * DF], BF16, tag="scr", name="scr")
                v_b = bass.AP(tensor=v_all.tensor, offset=v_all[:].offset,
                              ap=[v_all[:].ap[0], [3 * DF, HI], [0, LCH], [1, 3 * DF]])
                nc.vector.tensor_tensor(out=scr[:], in0=in0, in1=v_b, op=ALU.mult)
                # f-reduction 96->3 as in-place bf16 half-add tree (2x each),
                # then one tiny 1x reduce over the last 3.
                for w in (48, 24, 12, 6, 3):
                    nc.vector.tensor_tensor(
                        out=tree_view(scr, w), in0=tree_view(scr, w),
                        in1=tree_view(scr, w, off_elems=w), op=ALU.add)
                sred = ppool.tile([P, HI, LCH], BF16, tag="sred", name="sred")
                with nc.allow_low_precision(reason="score kept in bf16 anyway"):
                    nc.vector.tensor_reduce(out=sred[:], in_=tree_view(scr, 3),
                                            axis=AX.X, op=ALU.add)
                # s = sred + c, written duplicated x2 so the browse mult sees
                # a packed step-1 innermost pair on every operand (DVE 2x)
                ssl16 = ppool.tile([P, HI, LCH, 2], BF16, tag="ssl16", name="ssl16")
                sr = sred[:]
                sr_dup = bass.AP(tensor=sr.tensor, offset=sr.offset,
                                 ap=[sr.ap[0], [LCH, HI], [1, LCH], [0, 2]])
                ca = c_t[:]
                c_dup = bass.AP(tensor=ca.tensor, offset=ca.offset,
                                ap=[ca.ap[0], [1, HI], [0, LCH], [0, 2]])
                nc.vector.tensor_tensor(out=ssl16[:], in0=sr_dup, in1=c_dup,
                                        op=ALU.add)
                ua = u_all[:]
                in0p = bass.AP(tensor=ua.tensor, offset=ua.offset,
                               ap=[ua.ap[0], [3 * DF, HI], [6 * DF, LCH],
                                   [2, 48], [1, 2]])
                sa = ssl16[:]
                s_b = bass.AP(tensor=sa.tensor, offset=sa.offset,
                              ap=[sa.ap[0], [2 * LCH, HI], [2, LCH],
                                  [0, 48], [1, 2]])
                nc.vector.tensor_tensor(out=in0p, in0=in0p, in1=s_b, op=ALU.mult)
                # l-reduction 40->5 as in-place bf16 half-add tree. Free
                # offset of (l, hi, k, f) in u_all is l*192+hi*96+k*32+f, so
                # all operands stay packed bf16 (DVE 2x).
                def l_view(nl, off_l):
                    return bass.AP(tensor=ua.tensor,
                                   offset=ua.offset + off_l * 6 * DF,
                                   ap=[ua.ap[0], [3 * DF, HI],
                                       [6 * DF, nl], [1, 3 * DF]])
                for nl in (20, 10):
                    nc.vector.tensor_tensor(out=l_view(nl, 0), in0=l_view(nl, 0),
                                            in1=l_view(nl, nl), op=ALU.add)
                if ci == 0:
                    nc.vector.tensor_tensor(out=bacc16[:], in0=l_view(5, 0),
                                            in1=l_view(5, 5), op=ALU.add)
                else:
                    nc.vector.tensor_tensor(out=l_view(5, 0), in0=l_view(5, 0),
                                            in1=l_view(5, 5), op=ALU.add)
                    nc.vector.tensor_tensor(out=bacc16[:], in0=bacc16[:],
                                            in1=l_view(5, 0), op=ALU.add)

            # final browse reduce over the 5 accumulated l-slots -> f32
            bacc_t = ones.tile([P, HI, 3 * DF], F32, tag="bacc", name="bacc")
            ba = bacc16[:]
            red_in = bass.AP(tensor=ba.tensor, offset=ba.offset,
                             ap=[ba.ap[0], [5 * 3 * DF, HI], [1, 3 * DF],
                                 [3 * DF, 5]])
            nc.vector.tensor_reduce(out=bacc_t[:], in_=red_in, axis=AX.X,
                                    op=ALU.add)
            # --- feat assembly [p, hi, 256] = [item(96) | browse(96) | user(64)] ---
            feat = ones.tile([P, HI, 256], F32)
            for k in range(3):
                nc.vector.tensor_copy(out=feat[:, :, DF * k:DF * k + DF], in_=qk[k][:, :, 0:DF])
            nc.vector.tensor_copy(out=feat[:, :, E:E + 3 * DF], in_=bacc_t[:])
            nc.vector.tensor_copy(out=feat[:, :, 2 * E:2 * E + U], in_=user_t[:])

            if debug:
                nc.sync.dma_start(out=dbg_feat[:, :, :], in_=feat[:])
            # --- transpose feat -> featT [f, c2, b] via PE ---
            featT = ones.tile([P, 2, BS], F32)
            for hi in range(HI):
                for c2 in range(2):
                    pst = psum.tile([P, P], F32, tag="pst")
                    nc.tensor.transpose(out=pst[:], in_=feat[:, hi, c2 * P:(c2 + 1) * P],
                                        identity=ident[:])
                    nc.vector.tensor_copy(out=featT[:, c2, hi * P:(hi + 1) * P], in_=pst[:])

            # --- MLP layer 1: x1[h, b], h split in 2x100 ---
            x1 = [psum.tile([HS, BS], F32, tag=f"x1_{s}", name=f"x1_{s}", bufs=1) for s in range(2)]
            for s in range(2):
                for c2 in range(2):
                    nc.tensor.matmul(x1[s][:], w1t_t[:, c2, s * HS:(s + 1) * HS],
                                     featT[:, c2, :], start=(c2 == 0), stop=(c2 == 1))
            # add bias b1 (dice1[...,0]) then stats
            st1 = ones.tile([P, 4], F32)
            nc.vector.memset(st1[:], 0.0)
            x1d = ones.tile([HS, 2, BS], F32)
            sq = x1d
            for s in range(2):
                nc.vector.scalar_tensor_tensor(out=x1[s][:], in0=x1[s][:], scalar=1.0,
                                               in1=_bcast_col(d1_t[0:HS, s, 0:1], BS),
                                               op0=ALU.mult, op1=ALU.add)
                nc.vector.tensor_reduce(out=st1[0:HS, s:s + 1], in_=x1[s][:], axis=AX.X, op=ALU.add)
                nc.scalar.activation(out=sq[:, s, :], in_=x1[s][:], func=ACT.Square,
                                     accum_out=st1[0:HS, 2 + s:3 + s])
            if debug:
                nc.sync.dma_start(out=dbg_st1[:, :], in_=st1[:])
                x1dbg = small.tile([HS, 2, BS], F32, tag="x1dbg")
                for s in range(2):
                    nc.vector.tensor_copy(out=x1dbg[:, s, :], in_=x1[s][:])
                nc.sync.dma_start(out=dbg_x1[:, :, :], in_=x1dbg[:])
            red1 = ones.tile([P, 4], F32)
            if use_cc:
                nc.sync.dma_start(out=cc1_in[:, :], in_=st1[:])
                nc.gpsimd.collective_compute(
                    "AllReduce", ALU.add, replica_groups=groups,
                    ins=[cc1_in[:, :]], outs=[cc1_out[:, :]])
                nc.sync.dma_start(out=red1, in_=cc1_out[:, :])
            else:
                nc.vector.tensor_scalar(out=red1[:], in0=st1[:], scalar1=float(n_cores),
                                        scalar2=None, op0=ALU.mult)
            if debug:
                nc.sync.dma_start(out=dbg_red1[:, :], in_=red1[:])

            # --- Dice 1 + layer 2 ---
            x2 = psum.tile([H2, BS], F32, tag="x2", bufs=1)
            for s in range(2):
                mean = small.tile([HS, 1], F32, tag="mean")
                nc.vector.tensor_scalar(out=mean[:], in0=red1[0:HS, s:s + 1],
                                        scalar1=1.0 / B, scalar2=None, op0=ALU.mult)
                var = small.tile([HS, 1], F32, tag="var")
                nc.vector.tensor_tensor(out=var[:], in0=mean[:], in1=mean[:], op=ALU.mult)
                nc.vector.scalar_tensor_tensor(out=var[:], in0=var[:], scalar=-float(B),
                                               in1=red1[0:HS, 2 + s:3 + s], op0=ALU.mult, op1=ALU.add)
                nc.vector.tensor_scalar(out=var[:], in0=var[:], scalar1=1.0 / (B - 1),
                                        scalar2=EPS, op0=ALU.mult, op1=ALU.add)
                rstd = small.tile([HS, 1], F32, tag="rstd")
                nc.scalar.sqrt(out=rstd[:], in_=var[:])
                nc.vector.reciprocal(out=rstd[:], in_=rstd[:])
                scl = small.tile([HS, 1], F32, tag="scl")
                nc.vector.tensor_tensor(out=scl[:], in0=d1_t[0:HS, s, 2:3], in1=rstd[:], op=ALU.mult)
                bia = small.tile([HS, 1], F32, tag="bia")
                nc.vector.tensor_tensor(out=bia[:], in0=mean[:], in1=scl[:], op=ALU.mult)
                nc.vector.tensor_tensor(out=bia[:], in0=d1_t[0:HS, s, 3:4], in1=bia[:], op=ALU.subtract)
                # p = sigmoid(xn), xn = x1*scl + bia
                psig = small.tile([HS, BS], F32, tag="psig")
                nc.scalar.activation(out=psig[:], in_=x1[s][:], func=ACT.Sigmoid,
                                     bias=bia[:], scale=scl[:])
                xn = small.tile([HS, BS], F32, tag="xn")
                nc.vector.scalar_tensor_tensor(out=xn[:], in0=x1[s][:], scalar=scl[:],
                                               in1=_bcast_col(bia[:], BS), op0=ALU.mult, op1=ALU.add)
                # gate = p*(1-alpha) + alpha ; x1d = xn * gate
                oma = small.tile([HS, 1], F32, tag="oma")
                nc.vector.tensor_scalar(out=oma[:], in0=d1_t[0:HS, s, 1:2], scalar1=-1.0,
                                        scalar2=1.0, op0=ALU.mult, op1=ALU.add)
                gate = small.tile([HS, BS], F32, tag="gate")
                nc.vector.scalar_tensor_tensor(out=gate[:], in0=psig[:], scalar=oma[:],
                                               in1=_bcast_col(d1_t[0:HS, s, 1:2], BS),
                                               op0=ALU.mult, op1=ALU.add)
                nc.vector.tensor_tensor(out=x1d[:, s, :], in0=xn[:], in1=gate[:], op=ALU.mult)
                nc.tensor.matmul(x2[:], w2t_t[0:HS, s, :], x1d[:, s, :],
                                 start=(s == 0), stop=(s == 1))

            if debug:
                nc.sync.dma_start(out=dbg_x1d[:, :, :], in_=x1d[:])
            # --- stats 2 + Dice 2 + layer 3 ---
            st2 = ones.tile([P, 2], F32)
            nc.vector.memset(st2[:], 0.0)
            nc.vector.scalar_tensor_tensor(out=x2[:], in0=x2[:], scalar=1.0,
                                           in1=_bcast_col(d2_t[0:H2, 0:1], BS),
                                           op0=ALU.mult, op1=ALU.add)
            nc.vector.tensor_reduce(out=st2[0:H2, 0:1], in_=x2[:], axis=AX.X, op=ALU.add)
            x2d = small.tile([H2, BS], F32, tag="x2d")
            nc.scalar.activation(out=x2d[:], in_=x2[:], func=ACT.Square,
                                 accum_out=st2[0:H2, 1:2])
            red2 = ones.tile([P, 2], F32)
            if use_cc:
                nc.sync.dma_start(out=cc2_in[:, :], in_=st2[:])
                nc.gpsimd.collective_compute(
                    "AllReduce", ALU.add, replica_groups=groups,
                    ins=[cc2_in[:, :]], outs=[cc2_out[:, :]])
                nc.sync.dma_start(out=red2, in_=cc2_out[:, :])
            else:
                nc.vector.tensor_scalar(out=red2[:], in0=st2[:], scalar1=float(n_cores),
                                        scalar2=None, op0=ALU.mult)

            mean = small.tile([H2, 1], F32, tag="mean2")
            nc.vector.tensor_scalar(out=mean[:], in0=red2[0:H2, 0:1], scalar1=1.0 / B,
                                    scalar2=None, op0=ALU.mult)
            var = small.tile([H2, 1], F32, tag="var2")
            nc.vector.tensor_tensor(out=var[:], in0=mean[:], in1=mean[:], op=ALU.mult)
            nc.vector.scalar_tensor_tensor(out=var[:], in0=var[:], scalar=-float(B),
                                           in1=red2[0:H2, 1:2], op0=ALU.mult, op1=ALU.add)
            nc.vector.tensor_scalar(out=var[:], in0=var[:], scalar1=1.0 / (B - 1),
                                    scalar2=EPS, op0=ALU.mult, op1=ALU.add)
            rstd = small.tile([H2, 1], F32, tag="rstd2")
            nc.scalar.sqrt(out=rstd[:], in_=var[:])
            nc.vector.reciprocal(out=rstd[:], in_=rstd[:])
            scl = small.tile([H2, 1], F32, tag="scl2")
            nc.vector.tensor_tensor(out=scl[:], in0=d2_t[0:H2, 2:3], in1=rstd[:], op=ALU.mult)
            bia = small.tile([H2, 1], F32, tag="bia2")
            nc.vector.tensor_tensor(out=bia[:], in0=mean[:], in1=scl[:], op=ALU.mult)
            nc.vector.tensor_tensor(out=bia[:], in0=d2_t[0:H2, 3:4], in1=bia[:], op=ALU.subtract)
            psig = small.tile([H2, BS], F32, tag="psig2")
            nc.scalar.activation(out=psig[:], in_=x2[:], func=ACT.Sigmoid, bias=bia[:], scale=scl[:])
            xn = small.tile([H2, BS], F32, tag="xn2")
            nc.vector.scalar_tensor_tensor(out=xn[:], in0=x2[:], scalar=scl[:],
                                           in1=_bcast_col(bia[:], BS), op0=ALU.mult, op1=ALU.add)
            oma = small.tile([H2, 1], F32, tag="oma2")
            nc.vector.tensor_scalar(out=oma[:], in0=d2_t[0:H2, 1:2], scalar1=-1.0,
                                    scalar2=1.0, op0=ALU.mult, op1=ALU.add)
            gate = small.tile([H2, BS], F32, tag="gate2")
            nc.vector.scalar_tensor_tensor(out=gate[:], in0=psig[:], scalar=oma[:],
                                           in1=_bcast_col(d2_t[0:H2, 1:2], BS),
                                           op0=ALU.mult, op1=ALU.add)
            nc.vector.tensor_tensor(out=x2d[:], in0=xn[:], in1=gate[:], op=ALU.mult)

            x3 = psum.tile([1, BS], F32, tag="x3", bufs=1)
            nc.tensor.matmul(x3[:], w3t_t[0:H2, :], x2d[:], start=True, stop=True)
            yt = small.tile([1, BS], F32, tag="yt")
            nc.scalar.activation(out=yt[:], in_=x3[:], func=ACT.Sigmoid, bias=be[0:1, 1:2])
            nc.sync.dma_start(out=y_out[:, :], in_=yt[:])

    nc.compile()
    return nc


def _wrap16(stream):
    """[n] int -> [128, n/16] int16: idx j at (16g + j%16, j//16), replicated over 8 groups."""
    n = stream.shape[0]
    w = stream.reshape(n // 16, 16).T.astype(np.int16)
    return np.tile(w, (8, 1))


_NC_CACHE = {}


def prep_in_maps(inputs):
    user = np.asarray(inputs["user"])
    item = np.asarray(inputs["item"])
    rec_his = np.asarray(inputs["rec_his"])
    t1 = np.asarray(inputs["table_i1"], np.float32)
    t2 = np.asarray(inputs["table_i2"], np.float32)
    t3 = np.asarray(inputs["table_i3"], np.float32)
    tu = np.ascontiguousarray(np.asarray(inputs["table_user"], np.float32))
    w_att1 = np.asarray(inputs["w_att1"], np.float32)
    b_att1 = np.asarray(inputs["b_att1"], np.float32)
    w_att2 = np.asarray(inputs["w_att2"], np.float32)
    b_att2 = np.asarray(inputs["b_att2"], np.float32)

    # collapse the linear attention MLP
    w_eff = (w_att2 @ w_att1)[0]                      # [384]
    wq, wu, wd, wm = (w_eff[0:96], w_eff[96:192], w_eff[192:288], w_eff[288:384])
    b_eff = float(w_att2[0] @ b_att1 + b_att2[0])
    watt = np.concatenate([wm, wu - wd, wq + wd])[None, :].astype(np.float32)

    tab = np.zeros((3 * V_ITEM, ES), np.float32)
    tab[0:V_ITEM, 0:DF] = t1
    tab[V_ITEM:2 * V_ITEM, 0:DF] = t2
    tab[2 * V_ITEM:, 0:DF] = t3
    import ml_dtypes
    tab16 = np.zeros((3 * V_ITEM + 8, 2 * ES), ml_dtypes.bfloat16)
    tab16[0:V_ITEM, 0:DF] = t1
    tab16[V_ITEM:2 * V_ITEM, 0:DF] = t2
    tab16[2 * V_ITEM:3 * V_ITEM, 0:DF] = t3
    ZROW = 3 * V_ITEM                                 # all-zero row: mask sink

    w1 = np.asarray(inputs["w1"], np.float32)
    w1t = w1.T.reshape(2, P, H1).transpose(1, 0, 2).copy()          # [128, 2, 200]
    d1 = np.stack([inputs["b1"], inputs["a1"], inputs["g1"], inputs["be1"]],
                  -1).astype(np.float32)                             # [200, 4]
    dice1 = np.zeros((P, 2, 4), np.float32)
    dice1[0:HS] = d1.reshape(2, HS, 4).transpose(1, 0, 2)
    w2 = np.asarray(inputs["w2"], np.float32)
    w2t = np.zeros((P, 2, H2), np.float32)
    w2t[0:HS] = w2.T.reshape(2, HS, H2).transpose(1, 0, 2)
    d2 = np.stack([inputs["b2"], inputs["a2"], inputs["g2"], inputs["be2"]],
                  -1).astype(np.float32)                             # [80, 4]
    dice2 = np.zeros((P, 4), np.float32)
    dice2[0:H2] = d2
    w3t = np.zeros((P, 1), np.float32)
    w3t[0:H2, 0] = np.asarray(inputs["w3"], np.float32)[0]
    b3 = float(np.asarray(inputs["b3"], np.float32)[0])
    beff_arr = np.array([[b_eff, b3]], np.float32)
    in_maps = []
    for c in range(N_CORES):
        bsl = slice(c * BS, (c + 1) * BS)
        rec = rec_his[bsl].reshape(HI, P, L, 3)       # [hi, p, l, k]
        msk = rec[:, :, :, 0:1] == 0                  # [hi, p, l, 1] True -> masked
        offs = np.arange(3, dtype=np.int64) * V_ITEM
        vals = np.where(msk, ZROW, rec + offs)        # [hi, p, l, k]
        stream = vals.transpose(2, 0, 3, 1).reshape(-1).astype(np.int64)  # [l, hi, k, p]
        g_idx = _wrap16(stream)
        itm = item[bsl].reshape(HI, P, 3)
        q_idx = np.zeros((P, 3, BS // 16), np.int16)
        for k in range(3):
            q_idx[:, k, :] = _wrap16((itm[:, :, k].reshape(-1) + V_ITEM * k))
        uv = user[bsl, 0].reshape(HI, P)              # [hi, p]
        ustream = uv.reshape(-1).astype(np.int64)
        lo = np.where(ustream < 32768, ustream, 0)
        hi_ = np.where(ustream >= 32768, ustream - 32768, 0)
        usel = (uv >= 32768).astype(np.float32).T.copy()   # [p, hi]

        in_maps.append({
            "tab": tab, "tab16": tab16, "tu": tu, "g_idx": g_idx, "q_idx": q_idx,
            "u_lo": _wrap16(lo), "u_hi": _wrap16(hi_), "u_sel": usel,
            "watt": watt, "beff": beff_arr,
            "w1t": w1t, "dice1": dice1, "w2t": w2t, "dice2": dice2, "w3t": w3t,
            "ident": np.eye(P, dtype=np.float32),
        })

    return in_maps


def kernel(**inputs):
    if N_CORES not in _NC_CACHE:
        _NC_CACHE[N_CORES] = build_nc(N_CORES)
    nc = _NC_CACHE[N_CORES]
    in_maps = prep_in_maps(inputs)
    res = run_bass_kernel_spmd(nc, in_maps, core_ids=list(range(N_CORES)))
    out = np.concatenate([res.results[c]["y"][0] for c in range(N_CORES)])
    return out.astype(np.float32)

